# revision 12
# baseline (speedup 1.0000x reference)
"""Dense MoE forward for Trainium2: 8-core data-parallel SPMD Bass/Tile kernel.

Reference computation (per token row x[b, :], all experts dense):
    gates = softmax(x @ Wg + bg)                      # [B, E]
    h_e   = relu(x @ W1[e] + b1[e])                   # [B, H] per expert
    eo_e  = h_e @ W2[e] + b2[e]                       # [B, C]
    out   = sum_e gates[:, e] * eo_e                  # [B, C]

Strategy (per core, B_loc = B/8 tokens):
  - x is pre-cast to fp16 on host; DMA-transposed on load so D sits on SBUF
    partitions (xT tiles [128d x 512tok]).
  - Layer 1 feature-major: hT[e] = W1[e].T @ xT (PE, fp16 in / fp32 psum),
    relu+bias on ACT (per-partition bias), output fp16.
  - Gate logits token-major [tok, E]; softmax over free dim (exp on ACT,
    sum+reciprocal+normalize on DVE). Gate bias folded in via a K=1 matmul
    with a ones row.
  - Layer 2 token-major: eo[t·128:, :] = hT-slice.T @ W2[e] (+ b2 via K=1
    ones-row matmul), accumulated in PSUM.
  - Expert combine on DVE: acc = (eo_e * g_col_e) + acc  (scalar_tensor_tensor,
    per-partition gate scalar in token-major layout).

fp16 products are exact in fp32 accumulation, so matmul error is only the
fp16 input rounding (~2.4e-4 relative).
"""

import numpy as np

from concourse import bacc, bass, mybir, tile
from concourse.bass_utils import run_bass_kernel_spmd

B, D, C, E, H = 65536, 512, 101, 8, 256
N_CORES = 8
BL = B // N_CORES          # 8192 tokens per core
MACRO = 512                # tokens per macro tile (matmul moving-dim max)
SUB = 128                  # tokens per subtile (PE stationary free-dim max)
NSUB = MACRO // SUB        # 4
DK = D // 128              # 4 contraction chunks for layer 1 / gate
HJ = H // 128              # 2 contraction chunks for layer 2

F16 = mybir.dt.float16
F32 = mybir.dt.float32
AF = mybir.ActivationFunctionType
ALU = mybir.AluOpType


def build_nc(n_tokens: int = BL, repeat: int = 1) -> bass.Bass:
    """repeat > 1 re-runs the whole body in a hardware loop (same data,
    same outputs) — used only for timing measurements via wall-clock slope."""
    assert n_tokens % MACRO == 0
    n_macro = n_tokens // MACRO

    nc = bacc.Bacc("TRN2", debug=False)

    x = nc.dram_tensor("x", [n_tokens, D], F16, kind="ExternalInput").ap()
    w1 = nc.dram_tensor("w1", [128, DK, E, H], F16, kind="ExternalInput").ap()
    w2 = nc.dram_tensor("w2", [128, HJ, E, C], F16, kind="ExternalInput").ap()
    wg = nc.dram_tensor("wg", [128, DK, E], F16, kind="ExternalInput").ap()
    b1 = nc.dram_tensor("b1", [128, HJ, E], F32, kind="ExternalInput").ap()
    # b2 replicated across partitions (token-major bias add on DVE),
    # exp(bg) replicated across partitions and subtiles (gate reweighting).
    b2 = nc.dram_tensor("b2", [128, E, C], F32, kind="ExternalInput").ap()
    bg = nc.dram_tensor("bg", [128, NSUB, E], F32, kind="ExternalInput").ap()
    out = nc.dram_tensor("out", [n_tokens, C], F32, kind="ExternalOutput").ap()

    with tile.TileContext(nc) as tc:
        with (
            tc.tile_pool(name="wpool", bufs=1) as wpool,
            tc.tile_pool(name="xpool", bufs=2) as xpool,
            tc.tile_pool(name="hpool", bufs=2) as hpool,
            tc.tile_pool(name="gpool", bufs=2) as gpool,
            tc.tile_pool(name="apool", bufs=2) as apool,
            tc.tile_pool(name="pgp", bufs=1, space="PSUM") as pgp,
            tc.tile_pool(name="php", bufs=3, space="PSUM") as php,
            tc.tile_pool(name="peop", bufs=4, space="PSUM") as peop,
        ):
            # ---- persistent weights ----
            w1s = wpool.tile([128, DK, E, H], F16)
            w2s = wpool.tile([128, HJ, E, C], F16)
            wgs = wpool.tile([128, DK, E], F16)
            b1s = wpool.tile([128, HJ, E], F32)
            b2s = wpool.tile([128, E, C], F32)
            bgs = wpool.tile([128, NSUB, E], F32)

            nc.sync.dma_start(out=w1s[:], in_=w1)
            nc.sync.dma_start(out=w2s[:], in_=w2)
            nc.sync.dma_start(out=wgs[:], in_=wg)
            nc.sync.dma_start(out=b1s[:], in_=b1)
            nc.sync.dma_start(out=b2s[:], in_=b2)
            nc.sync.dma_start(out=bgs[:], in_=bg)

            import contextlib
            rep_ctx = (
                tc.For_i(0, repeat, 1) if repeat > 1
                else contextlib.nullcontext()
            )
            with rep_ctx:
                body(nc, tc, n_macro, x, out, w1s, w2s, wgs, b1s, b2s, bgs,
                     xpool, hpool, gpool, apool, pgp, php, peop)

    nc.compile()
    return nc


def body(nc, tc, n_macro, x, out, w1s, w2s, wgs, b1s, b2s, bgs,
         xpool, hpool, gpool, apool, pgp, php, peop):
    if True:
            for m in range(n_macro):
                t0 = m * MACRO

                # ---- xT: [128 d, MACRO tok] per d-chunk via DMA transpose ----
                xt = xpool.tile([128, DK, MACRO], F16, tag="xt")
                for k in range(DK):
                    nc.sync.dma_start_transpose(
                        xt[:, k], x[t0:t0 + MACRO, k * 128:(k + 1) * 128]
                    )

                # ---- gates (token-major logits, softmax over E) ----
                # u = exp(x @ Wg) * exp(bg); normalize by its row sum.
                pg = pgp.tile([128, NSUB, E], F32, tag="pg")
                for t in range(NSUB):
                    for k in range(DK):
                        nc.tensor.matmul(
                            pg[:, t],
                            lhsT=xt[:, k, t * SUB:(t + 1) * SUB],
                            rhs=wgs[:, k],
                            start=(k == 0),
                            stop=(k == DK - 1),
                        )
                u = gpool.tile([128, NSUB, E], F32, tag="u")
                nc.scalar.activation(u[:], pg[:], AF.Exp)
                u2 = gpool.tile([128, NSUB, E], F32, tag="u2")
                nc.vector.tensor_mul(u2[:], u[:], bgs[:])
                s = gpool.tile([128, NSUB], F32, tag="s")
                nc.vector.reduce_sum(s[:], u2[:], axis=mybir.AxisListType.X)
                r = gpool.tile([128, NSUB], F32, tag="r")
                nc.vector.reciprocal(r[:], s[:])
                g = gpool.tile([128, NSUB, E], F32, tag="g")
                for t in range(NSUB):
                    nc.vector.tensor_scalar_mul(g[:, t], u2[:, t], r[:, t:t + 1])

                acc = apool.tile([128, NSUB, C], F32, tag="acc")

                def emit_l2(e, ht):
                    """Layer 2 + gated combine for expert e (token-major).
                    acc += g_e * (h_e @ W2_e) + g_e * b2_e; the b2 term uses
                    the partition-replicated b2s tile on DVE (no PE matmul).
                    """
                    for t in range(NSUB):
                        peo = peop.tile([128, C], F32, tag="peo", name="peo")
                        for j in range(HJ):
                            nc.tensor.matmul(
                                peo[:],
                                lhsT=ht[:, j, t * SUB:(t + 1) * SUB],
                                rhs=w2s[:, j, e],
                                start=(j == 0),
                                stop=(j == HJ - 1),
                            )
                        if e == 0:
                            nc.vector.tensor_scalar_mul(
                                acc[:, t], peo[:], g[:, t, e:e + 1]
                            )
                        else:
                            nc.vector.scalar_tensor_tensor(
                                acc[:, t], peo[:], g[:, t, e:e + 1], acc[:, t],
                                op0=ALU.mult, op1=ALU.add,
                            )
                        nc.vector.scalar_tensor_tensor(
                            acc[:, t], b2s[:, e], g[:, t, e:e + 1], acc[:, t],
                            op0=ALU.mult, op1=ALU.add,
                        )

                # ---- experts: L1(e) emitted before L2(e-1) so the PE always
                # has independent matmul work while ACT runs relu(e). ----
                pending = None
                for e in range(E):
                    phs = [
                        php.tile([128, MACRO], F32, tag="ph", name="ph")
                        for _ in range(HJ)
                    ]
                    for j in range(HJ):
                        for k in range(DK):
                            nc.tensor.matmul(
                                phs[j][:],
                                lhsT=w1s[:, k, e, j * 128:(j + 1) * 128],
                                rhs=xt[:, k],
                                start=(k == 0),
                                stop=(k == DK - 1),
                            )
                    ht = hpool.tile([128, HJ, MACRO], F16, tag="ht", name="ht")
                    for j in range(HJ):
                        nc.scalar.activation(
                            ht[:, j], phs[j][:], AF.Relu, bias=b1s[:, j, e:e + 1]
                        )
                    if pending is not None:
                        emit_l2(*pending)
                    pending = (e, ht)
                emit_l2(*pending)

                # ---- store ----
                for t in range(NSUB):
                    nc.sync.dma_start(
                        out=out[t0 + t * SUB:t0 + (t + 1) * SUB, :],
                        in_=acc[:, t],
                    )


def _prep_weights(W1, b1, W2, b2, Wg, bg):
    w1p = np.ascontiguousarray(
        W1.astype(np.float16).transpose(1, 0, 2).reshape(DK, 128, E, H)
        .transpose(1, 0, 2, 3)
    )
    w2p = np.ascontiguousarray(
        W2.astype(np.float16).transpose(1, 0, 2).reshape(HJ, 128, E, C)
        .transpose(1, 0, 2, 3)
    )
    wgp = np.ascontiguousarray(
        Wg.astype(np.float16).reshape(DK, 128, E).transpose(1, 0, 2)
    )
    b1p = np.ascontiguousarray(
        b1.astype(np.float32).T.reshape(HJ, 128, E).transpose(1, 0, 2)
    )
    b2p = np.ascontiguousarray(
        np.broadcast_to(b2.astype(np.float32), (128, E, C))
    )
    bgp = np.ascontiguousarray(np.broadcast_to(
        np.exp(bg).astype(np.float32), (128, NSUB, E)
    ))
    return w1p, w2p, wgp, b1p, b2p, bgp


_CACHE: dict = {}


def kernel(x, W1, b1, W2, b2, Wg, bg, _trace=False):
    x = np.asarray(x, dtype=np.float32)
    W1 = np.asarray(W1, dtype=np.float32)
    b1 = np.asarray(b1, dtype=np.float32)
    W2 = np.asarray(W2, dtype=np.float32)
    b2 = np.asarray(b2, dtype=np.float32)
    Wg = np.asarray(Wg, dtype=np.float32)
    bg = np.asarray(bg, dtype=np.float32)

    if "nc" not in _CACHE:
        _CACHE["nc"] = build_nc()
    nc = _CACHE["nc"]

    x16 = x.astype(np.float16)
    w1p, w2p, wgp, b1p, b2p, bgp = _prep_weights(W1, b1, W2, b2, Wg, bg)

    in_maps = [
        {
            "x": x16[i * BL:(i + 1) * BL],
            "w1": w1p, "w2": w2p, "wg": wgp,
            "b1": b1p, "b2": b2p, "bg": bgp,
        }
        for i in range(N_CORES)
    ]
    try:
        res = run_bass_kernel_spmd(
            nc, in_maps, core_ids=list(range(N_CORES)), trace=_trace
        )
    except ModuleNotFoundError:
        # NTFF profile hook unavailable in this container — run untraced.
        res = run_bass_kernel_spmd(
            nc, in_maps, core_ids=list(range(N_CORES)), trace=False
        )
    out = np.concatenate(
        [res.results[i]["out"] for i in range(N_CORES)], axis=0
    )
    if _trace:
        _CACHE["last_result"] = res
    return out


# revision 13
# speedup vs baseline: 9727.8185x; 9727.8185x over previous
"""Dense MoE forward for Trainium2: 8-core data-parallel SPMD Bass/Tile kernel.

Reference computation (per token row x[b, :], all experts dense):
    gates = softmax(x @ Wg + bg)                      # [B, E]
    h_e   = relu(x @ W1[e] + b1[e])                   # [B, H] per expert
    eo_e  = h_e @ W2[e] + b2[e]                       # [B, C]
    out   = sum_e gates[:, e] * eo_e                  # [B, C]

Strategy (per core, B_loc = B/8 tokens):
  - x is pre-cast to fp16 on host; DMA-transposed on load so D sits on SBUF
    partitions (xT tiles [128d x 512tok]).
  - Layer 1 feature-major: hT[e] = W1[e].T @ xT (PE, fp16 in / fp32 psum),
    relu+bias on ACT (per-partition bias), output fp16.
  - Gate logits token-major [tok, E]; softmax over free dim (exp on ACT,
    sum+reciprocal+normalize on DVE). Gate bias applied multiplicatively:
    u = exp(logits) * exp(bg), with exp(bg) host-replicated across partitions.
  - Layer 2 token-major: eo[t*128:, :] = hT-slice.T @ W2[e], accumulated in
    PSUM. No bias matmuls: b2 is host-replicated across partitions and enters
    through the combine (small-N K=1 matmuls measured ~as expensive as real
    work, so they were eliminated).
  - Expert combine on DVE, per (expert, subtile): two fused ops
    acc = (eo_e * g_col_e) + acc; acc = (b2_e * g_col_e) + acc
    (scalar_tensor_tensor; the gate is a per-partition [128,1] scalar in
    token-major layout).

fp16 products are exact in fp32 accumulation, so matmul error is only the
fp16 input rounding (~2.4e-4 relative).
"""

import numpy as np

from concourse import bacc, bass, mybir, tile
from concourse.bass_utils import run_bass_kernel_spmd

B, D, C, E, H = 65536, 512, 101, 8, 256
N_CORES = 8
BL = B // N_CORES          # 8192 tokens per core
MACRO = 512                # tokens per macro tile (matmul moving-dim max)
SUB = 128                  # tokens per subtile (PE stationary free-dim max)
NSUB = MACRO // SUB        # 4
DK = D // 128              # 4 contraction chunks for layer 1 / gate
HJ = H // 128              # 2 contraction chunks for layer 2

F16 = mybir.dt.float16
F32 = mybir.dt.float32
AF = mybir.ActivationFunctionType
ALU = mybir.AluOpType


def build_nc(n_tokens: int = BL, repeat: int = 1) -> bass.Bass:
    """repeat > 1 re-runs the whole body in a hardware loop (same data,
    same outputs) — used only for timing measurements via wall-clock slope."""
    assert n_tokens % MACRO == 0
    n_macro = n_tokens // MACRO

    nc = bacc.Bacc("TRN2", debug=False)

    x = nc.dram_tensor("x", [n_tokens, D], F16, kind="ExternalInput").ap()
    w1 = nc.dram_tensor("w1", [128, DK, E, H], F16, kind="ExternalInput").ap()
    w2 = nc.dram_tensor("w2", [128, HJ, E, C], F16, kind="ExternalInput").ap()
    wg = nc.dram_tensor("wg", [128, DK, E], F16, kind="ExternalInput").ap()
    b1 = nc.dram_tensor("b1", [128, HJ, E], F32, kind="ExternalInput").ap()
    # b2 replicated across partitions (token-major bias add on DVE),
    # exp(bg) replicated across partitions and subtiles (gate reweighting).
    b2 = nc.dram_tensor("b2", [128, E, C], F32, kind="ExternalInput").ap()
    bg = nc.dram_tensor("bg", [128, NSUB, E], F32, kind="ExternalInput").ap()
    out = nc.dram_tensor("out", [n_tokens, C], F32, kind="ExternalOutput").ap()

    with tile.TileContext(nc) as tc:
        with (
            tc.tile_pool(name="wpool", bufs=1) as wpool,
            tc.tile_pool(name="xpool", bufs=2) as xpool,
            tc.tile_pool(name="hpool", bufs=2) as hpool,
            tc.tile_pool(name="gpool", bufs=2) as gpool,
            tc.tile_pool(name="apool", bufs=2) as apool,
            tc.tile_pool(name="pgp", bufs=1, space="PSUM") as pgp,
            tc.tile_pool(name="php", bufs=3, space="PSUM") as php,
            tc.tile_pool(name="peop", bufs=4, space="PSUM") as peop,
        ):
            # ---- persistent weights ----
            w1s = wpool.tile([128, DK, E, H], F16)
            w2s = wpool.tile([128, HJ, E, C], F16)
            wgs = wpool.tile([128, DK, E], F16)
            b1s = wpool.tile([128, HJ, E], F32)
            b2s = wpool.tile([128, E, C], F32)
            bgs = wpool.tile([128, NSUB, E], F32)

            nc.sync.dma_start(out=w1s[:], in_=w1)
            nc.sync.dma_start(out=w2s[:], in_=w2)
            nc.sync.dma_start(out=wgs[:], in_=wg)
            nc.sync.dma_start(out=b1s[:], in_=b1)
            nc.sync.dma_start(out=b2s[:], in_=b2)
            nc.sync.dma_start(out=bgs[:], in_=bg)

            import contextlib
            rep_ctx = (
                tc.For_i(0, repeat, 1) if repeat > 1
                else contextlib.nullcontext()
            )
            with rep_ctx:
                body(nc, tc, n_macro, x, out, w1s, w2s, wgs, b1s, b2s, bgs,
                     xpool, hpool, gpool, apool, pgp, php, peop)

    nc.compile()
    return nc


def body(nc, tc, n_macro, x, out, w1s, w2s, wgs, b1s, b2s, bgs,
         xpool, hpool, gpool, apool, pgp, php, peop):
    if True:
            for m in range(n_macro):
                t0 = m * MACRO

                # ---- xT: [128 d, MACRO tok] per d-chunk via DMA transpose ----
                xt = xpool.tile([128, DK, MACRO], F16, tag="xt")
                for k in range(DK):
                    nc.sync.dma_start_transpose(
                        xt[:, k], x[t0:t0 + MACRO, k * 128:(k + 1) * 128]
                    )

                # ---- gates (token-major logits, softmax over E) ----
                # u = exp(x @ Wg) * exp(bg); normalize by its row sum.
                pg = pgp.tile([128, NSUB, E], F32, tag="pg")
                for t in range(NSUB):
                    for k in range(DK):
                        nc.tensor.matmul(
                            pg[:, t],
                            lhsT=xt[:, k, t * SUB:(t + 1) * SUB],
                            rhs=wgs[:, k],
                            start=(k == 0),
                            stop=(k == DK - 1),
                        )
                u = gpool.tile([128, NSUB, E], F32, tag="u")
                nc.scalar.activation(u[:], pg[:], AF.Exp)
                u2 = gpool.tile([128, NSUB, E], F32, tag="u2")
                nc.vector.tensor_mul(u2[:], u[:], bgs[:])
                s = gpool.tile([128, NSUB], F32, tag="s")
                nc.vector.reduce_sum(s[:], u2[:], axis=mybir.AxisListType.X)
                r = gpool.tile([128, NSUB], F32, tag="r")
                nc.vector.reciprocal(r[:], s[:])
                g = gpool.tile([128, NSUB, E], F32, tag="g")
                for t in range(NSUB):
                    nc.vector.tensor_scalar_mul(g[:, t], u2[:, t], r[:, t:t + 1])

                acc = apool.tile([128, NSUB, C], F32, tag="acc")

                def emit_l2(e, ht):
                    """Layer 2 + gated combine for expert e (token-major).
                    acc += g_e * (h_e @ W2_e) + g_e * b2_e; the b2 term uses
                    the partition-replicated b2s tile on DVE (no PE matmul).
                    """
                    for t in range(NSUB):
                        peo = peop.tile([128, C], F32, tag="peo", name="peo")
                        for j in range(HJ):
                            nc.tensor.matmul(
                                peo[:],
                                lhsT=ht[:, j, t * SUB:(t + 1) * SUB],
                                rhs=w2s[:, j, e],
                                start=(j == 0),
                                stop=(j == HJ - 1),
                            )
                        if e == 0:
                            nc.vector.tensor_scalar_mul(
                                acc[:, t], peo[:], g[:, t, e:e + 1]
                            )
                        else:
                            nc.vector.scalar_tensor_tensor(
                                acc[:, t], peo[:], g[:, t, e:e + 1], acc[:, t],
                                op0=ALU.mult, op1=ALU.add,
                            )
                        nc.vector.scalar_tensor_tensor(
                            acc[:, t], b2s[:, e], g[:, t, e:e + 1], acc[:, t],
                            op0=ALU.mult, op1=ALU.add,
                        )

                # ---- experts: L1(e) emitted before L2(e-1) so the PE always
                # has independent matmul work while ACT runs relu(e). ----
                pending = None
                for e in range(E):
                    phs = [
                        php.tile([128, MACRO], F32, tag="ph", name="ph")
                        for _ in range(HJ)
                    ]
                    for j in range(HJ):
                        for k in range(DK):
                            nc.tensor.matmul(
                                phs[j][:],
                                lhsT=w1s[:, k, e, j * 128:(j + 1) * 128],
                                rhs=xt[:, k],
                                start=(k == 0),
                                stop=(k == DK - 1),
                            )
                    ht = hpool.tile([128, HJ, MACRO], F16, tag="ht", name="ht")
                    for j in range(HJ):
                        nc.scalar.activation(
                            ht[:, j], phs[j][:], AF.Relu, bias=b1s[:, j, e:e + 1]
                        )
                    if pending is not None:
                        emit_l2(*pending)
                    pending = (e, ht)
                emit_l2(*pending)

                # ---- store ----
                for t in range(NSUB):
                    nc.sync.dma_start(
                        out=out[t0 + t * SUB:t0 + (t + 1) * SUB, :],
                        in_=acc[:, t],
                    )


def _prep_weights(W1, b1, W2, b2, Wg, bg):
    w1p = np.ascontiguousarray(
        W1.astype(np.float16).transpose(1, 0, 2).reshape(DK, 128, E, H)
        .transpose(1, 0, 2, 3)
    )
    w2p = np.ascontiguousarray(
        W2.astype(np.float16).transpose(1, 0, 2).reshape(HJ, 128, E, C)
        .transpose(1, 0, 2, 3)
    )
    wgp = np.ascontiguousarray(
        Wg.astype(np.float16).reshape(DK, 128, E).transpose(1, 0, 2)
    )
    b1p = np.ascontiguousarray(
        b1.astype(np.float32).T.reshape(HJ, 128, E).transpose(1, 0, 2)
    )
    b2p = np.ascontiguousarray(
        np.broadcast_to(b2.astype(np.float32), (128, E, C))
    )
    bgp = np.ascontiguousarray(np.broadcast_to(
        np.exp(bg).astype(np.float32), (128, NSUB, E)
    ))
    return w1p, w2p, wgp, b1p, b2p, bgp


_CACHE: dict = {}


def kernel(x, W1, b1, W2, b2, Wg, bg, _trace=False):
    x = np.asarray(x, dtype=np.float32)
    W1 = np.asarray(W1, dtype=np.float32)
    b1 = np.asarray(b1, dtype=np.float32)
    W2 = np.asarray(W2, dtype=np.float32)
    b2 = np.asarray(b2, dtype=np.float32)
    Wg = np.asarray(Wg, dtype=np.float32)
    bg = np.asarray(bg, dtype=np.float32)

    if "nc" not in _CACHE:
        _CACHE["nc"] = build_nc()
    nc = _CACHE["nc"]

    x16 = x.astype(np.float16)
    w1p, w2p, wgp, b1p, b2p, bgp = _prep_weights(W1, b1, W2, b2, Wg, bg)

    in_maps = [
        {
            "x": x16[i * BL:(i + 1) * BL],
            "w1": w1p, "w2": w2p, "wg": wgp,
            "b1": b1p, "b2": b2p, "bg": bgp,
        }
        for i in range(N_CORES)
    ]
    try:
        res = run_bass_kernel_spmd(
            nc, in_maps, core_ids=list(range(N_CORES)), trace=_trace
        )
    except ModuleNotFoundError:
        # NTFF profile hook unavailable in this container — run untraced.
        res = run_bass_kernel_spmd(
            nc, in_maps, core_ids=list(range(N_CORES)), trace=False
        )
    out = np.concatenate(
        [res.results[i]["out"] for i in range(N_CORES)], axis=0
    )
    if _trace:
        _CACHE["last_result"] = res
    return out
